# revision 31
# baseline (speedup 1.0000x reference)
"""Trainium2 Bass kernel for the nn_LSTMCell problem.

Strategy: data-parallel over the batch dim (4096 -> 8 cores x 512), weights
replicated. All on-chip compute happens in "transposed" orientation
(hidden on PSUM partitions, batch on the free dim) so every matmul operand
can be DMA'd in its natural, contiguous layout:

    gate.T[h, b] = sum_k W.T[k, h] * act.T[k, b]
    matmul(out[M=h128, N=b512], lhsT=WT_tile[K=k128, M=h128], rhs=actT[K=k128, N=b512])

Mixed precision: most gate matmuls run in fp8(e4m3) with DoubleRow perf mode
(2 k-tiles per matmul, ~2x PE throughput); the error-critical matrices (the
tanh'd cell-candidate gate, and optionally parts of the output gate) stay in
bf16. All operands are pre-scaled on the host (W*256, act*16) so fp8 values
sit in the normal range; the 2^-12 descale is folded into the gate activation
instruction. PSUM accumulation is fp32 throughout, as is all elementwise math.

Per matrix, CONFIG[name] = number of 128-wide k-tiles (out of 16) computed in
fp8-DoubleRow; the first 16-n8 k-tiles run in bf16. Both forms accumulate
into the same PSUM (uniform operand scaling makes that legal).

Per core:
  phase 1: for each of 16 h-tiles: i/f/g gate matmuls, sigmoid/tanh,
           c1 = f*c0 + i*tanh(g)  -> c1 (fp32, kept in SBUF + DMA'd out),
           c1 scaled+cast to fp8 (matmul operand for the o gate).
  phase 2: for each of 16 h-tiles: o gate matmuls (incl. W_co @ c1.T),
           o = sigmoid(...), h1 = o * tanh(c1), DMA out.
"""

import numpy as np
import ml_dtypes
from contextlib import ExitStack

BF = ml_dtypes.bfloat16
F8 = ml_dtypes.float8_e4m3

N_CORES = 8
P = 128          # partition dim / k-tile size / m-tile size
BATCH = 4096
IN_DIM = 2048
HID = 2048
B = BATCH // N_CORES          # 512, batch per core = matmul free dim
NK = 2048 // P                # 16, k-tiles per weight matrix contraction
MT = HID // P                 # 16, output h-tiles

W_NAMES = ["ii", "hi", "if_", "hf", "cf", "ic", "hc", "io", "ho", "co"]

# k-tiles (of 16) per matrix computed in fp8-DoubleRow; rest in bf16.
# The g gate (ic/hc, goes through tanh into c1) dominates the fp8 error and
# stays bf16; io/ho are the next-largest contributors.
CONFIG = {
    "ii": 16, "hi": 16,
    "if_": 16, "hf": 16, "cf": 16,
    "ic": 0, "hc": 0,
    "io": 16, "ho": 16, "co": 16,
}

SW = 256.0   # host-side weight scale (all matrices, both dtypes)
SA = 16.0    # host-side activation scale (x/h/c0 and on-device c1)
INV_S = 1.0 / (SW * SA)

# which activation operand forms are needed on device
_X_MATS = ("ii", "if_", "ic", "io")
_H_MATS = ("hi", "hf", "hc", "ho")
NEED_X8 = any(CONFIG[n] > 0 for n in _X_MATS)
NEED_X16 = any(CONFIG[n] < NK for n in _X_MATS)
NEED_H8 = any(CONFIG[n] > 0 for n in _H_MATS)
NEED_H16 = any(CONFIG[n] < NK for n in _H_MATS)
NEED_C8 = CONFIG["cf"] > 0
NEED_C16 = CONFIG["cf"] < NK
NEED_C18 = CONFIG["co"] > 0
NEED_C116 = CONFIG["co"] < NK


def _build(p, nk, mt, b):
    import concourse.tile as tile
    from concourse import bacc, mybir

    bf16, f32 = mybir.dt.bfloat16, mybir.dt.float32
    f8 = mybir.dt.float8e4
    Sig = mybir.ActivationFunctionType.Sigmoid
    Tanh = mybir.ActivationFunctionType.Tanh
    Copy = mybir.ActivationFunctionType.Copy
    DR = mybir.MatmulPerfMode.DoubleRow

    nc = bacc.Bacc(
        "TRN2",
        target_bir_lowering=False,
        debug=False,
        num_devices=N_CORES,
    )

    def act_in(name, dt):
        return nc.dram_tensor(name, [p, nk, b], dt, kind="ExternalInput").ap()

    xT8 = act_in("xT8", f8) if NEED_X8 else None
    xT16 = act_in("xT16", bf16) if NEED_X16 else None
    hT8 = act_in("hT8", f8) if NEED_H8 else None
    hT16 = act_in("hT16", bf16) if NEED_H16 else None
    cT8 = act_in("cT8", f8) if NEED_C8 else None
    cT16 = act_in("cT16", bf16) if NEED_C16 else None
    c0T = nc.dram_tensor("c0T", [p, mt, b], f32, kind="ExternalInput").ap()
    bias = nc.dram_tensor("bias", [p, mt, 4], f32, kind="ExternalInput").ap()

    w8, w16 = {}, {}
    for n in W_NAMES:
        n8 = CONFIG[n]
        if n8 > 0:
            w8[n] = nc.dram_tensor(
                f"w8_{n}", [mt, p, n8, p], f8, kind="ExternalInput").ap()
        if n8 < nk:
            w16[n] = nc.dram_tensor(
                f"w16_{n}", [mt, p, nk - n8, p], bf16, kind="ExternalInput").ap()

    ogT = nc.dram_tensor("ogT", [p, mt, b], f32, kind="ExternalOutput").ap()
    h1T = nc.dram_tensor("h1T", [p, mt, b], f32, kind="ExternalOutput").ap()
    c1T = nc.dram_tensor("c1T", [p, mt, b], f32, kind="ExternalOutput").ap()

    with tile.TileContext(nc) as tc, ExitStack() as ctx:
        acts = ctx.enter_context(tc.tile_pool(name="acts", bufs=1))
        wpool = ctx.enter_context(tc.tile_pool(name="w", bufs=3))
        cpool = ctx.enter_context(tc.tile_pool(name="c0", bufs=2))
        tpool = ctx.enter_context(tc.tile_pool(name="temps", bufs=2))
        ppool = ctx.enter_context(tc.tile_pool(name="psum", bufs=8, space="PSUM"))

        # resident activation tensors. Loads go on the gpsimd/sync DMA issue
        # queues, split into chunks so the first matmuls — which only need the
        # first x chunks plus one weight slab — start early.
        CH = 4  # k-tiles per DMA chunk
        sb = {}
        loads = []
        # spread the activation preload over four DMA issue queues so the
        # early m-tiles (whose matmuls consume data as fast as it lands) are
        # not bottlenecked on a single queue's descriptor rate. x8 gets a
        # small leading chunk so the very first matmul can start early.
        for key, need, src, dt, eng, chunks in (
            ("x8", NEED_X8, xT8, f8, nc.gpsimd, (2, 2, 4, 4, 4)),
            ("x16", NEED_X16, xT16, bf16, nc.sync, (2, 2, 4, 4, 4)),
            ("h8", NEED_H8, hT8, f8, nc.gpsimd, (4, 4, 4, 4)),
            ("h16", NEED_H16, hT16, bf16, nc.sync, (4, 4, 4, 4)),
            ("c8", NEED_C8, cT8, f8, nc.gpsimd, (8, 8)),
            ("c16", NEED_C16, cT16, bf16, nc.sync, (8, 8)),
        ):
            if need:
                sb[key] = acts.tile([p, nk, b], dt, tag=key, name=key + "_sb")
                loads.append((src, sb[key], eng, chunks))
        bias_sb = acts.tile([p, mt, 4], f32, tag="bias")
        nc.scalar.dma_start(bias_sb[:], bias[:])
        for src, dst, eng, chunks in loads:
            c = 0
            for ch in chunks:
                eng.dma_start(dst[:, c:c + ch, :], src[:, c:c + ch, :])
                c += ch
        c1f_sb = acts.tile([p, mt, b], f32, tag="c1f")    # new cell state, fp32
        c18_sb = (acts.tile([p, mt, b], f8, tag="c18", name="c18_sb")
                  if NEED_C18 else None)
        c116_sb = (acts.tile([p, mt, b], bf16, tag="c116", name="c116_sb")
                   if NEED_C116 else None)

        def load_w(name, tag, m, chunks=1, eng=None):
            """Load this matrix's bf16 part and fp8 part; returns (t16, t8)."""
            t16 = t8 = None
            n8 = CONFIG[name]
            if n8 < nk:
                nkp = nk - n8
                t16 = wpool.tile([p, nkp, p], bf16, tag=tag + "b")
                step = max(1, nkp // chunks)
                for c in range(0, nkp, step):
                    (eng or nc.sync).dma_start(
                        t16[:, c:c + step], w16[name][m, :, c:c + step])
            if n8 > 0:
                t8 = wpool.tile([p, n8, p], f8, tag=tag + "a")
                step = max(1, n8 // chunks)
                for c in range(0, n8, step):
                    (eng or nc.sync).dma_start(
                        t8[:, c:c + step], w8[name][m, :, c:c + step])
            return t16, t8

        def emit_pair(pair, contribs, wts, ps):
            """Emit all matmuls for a pair of m-tiles, k-major with the pair
            innermost, so each activation k-chunk feeds both m-tiles' matmuls
            and the DMA demand rate per chunk is halved (matters during the
            ramp, where the first m-tiles otherwise outrun the preload).

            contribs: ordered list of (gate, matrix-name, act16, act8);
            wts[(name, m)] = (t16, t8); ps[(gate, m)] = psum tile.
            """
            tot = {}
            for gate, name, _, _ in contribs:
                n8 = CONFIG[name]
                tot[gate] = tot.get(gate, 0) + (nk - n8) + n8 // 2
            idx = {g: 0 for g in tot}
            for gate, name, a16, a8 in contribs:
                n8 = CONFIG[name]
                nb = nk - n8
                for ko in range(nb):
                    st = idx[gate] == 0
                    sp = idx[gate] == tot[gate] - 1
                    for mm_ in pair:
                        nc.tensor.matmul(
                            ps[(gate, mm_)][:], lhsT=wts[(name, mm_)][0][:, ko],
                            rhs=a16[:, ko], start=st, stop=sp,
                        )
                    idx[gate] += 1
                for ko2 in range(0, n8, 2):
                    st = idx[gate] == 0
                    sp = idx[gate] == tot[gate] - 1
                    for mm_ in pair:
                        nc.tensor.matmul(
                            ps[(gate, mm_)][:],
                            lhsT=wts[(name, mm_)][1][:, ko2:ko2 + 2],
                            rhs=a8[:, nb + ko2:nb + ko2 + 2],
                            start=st, stop=sp, perf_mode=DR,
                        )
                    idx[gate] += 1

        x16, x8 = sb.get("x16"), sb.get("x8")
        h16, h8 = sb.get("h16"), sb.get("h8")

        # ---- phase 1: i/f/g gates + new cell state, in m-tile pairs ----
        # x-term weights load (and matmul) first so the first m-tile's PE work
        # starts as soon as x chunks land, while h/c still stream in.
        P1_CONTRIBS = [
            ("i", "ii", x16, x8),
            ("f", "if_", x16, x8),
            ("g", "ic", x16, x8),
            ("i", "hi", h16, h8),
            ("f", "hf", h16, h8),
            ("g", "hc", h16, h8),
            ("f", "cf", sb.get("c16"), sb.get("c8")),
        ]
        # m0/m1 run as singletons: batching their 17 weight-slab issues
        # back-to-back trips the per-queue outstanding-DMA limit and
        # serializes the ramp. m2-m5 run as k-major pairs to halve the
        # activation demand rate while the preload is still streaming.
        p1_groups = [(0,), (1,), (2, 3), (4, 5)] + [(m,) for m in range(6, mt)]
        p2_pre = {}
        for pair in p1_groups:
            wts = {}
            for mm_ in pair:
                # the first m-tiles' slab issues go on the otherwise-idle
                # scalar engine queue: the sync/gpsimd queues are saturated
                # streaming the activation preload during the ramp.
                if mm_ == 0:
                    w_eng, ch = nc.scalar, 2
                elif mm_ in (1, 2):
                    w_eng, ch = nc.scalar, 1
                else:
                    w_eng, ch = None, 1
                for name, tag in (("ii", "w0"), ("if_", "w2"), ("ic", "w5"),
                                  ("hi", "w1"), ("hf", "w3"), ("hc", "w6"),
                                  ("cf", "w4")):
                    wts[(name, mm_)] = load_w(name, tag, mm_,
                                              chunks=ch, eng=w_eng)
                if mm_ == mt - 2:
                    # prefetch the first two phase-2 weight slab sets on the
                    # scalar queue so phase 2 starts without a DMA stall
                    for pm in (0, 1):
                        p2_pre[pm] = (load_w("io", "u0", pm, eng=nc.scalar),
                                      load_w("ho", "u1", pm, eng=nc.scalar),
                                      load_w("co", "u2", pm, eng=nc.scalar))

            ps = {}
            for gate in ("i", "f", "g"):
                for mm_ in pair:
                    ps[(gate, mm_)] = ppool.tile(
                        [p, b], f32, tag="ps", name=f"ps_{gate}{mm_}")
            emit_pair(pair, P1_CONTRIBS, wts, ps)

            for mm_ in pair:
                i_act = tpool.tile([p, b], f32, tag="i_act")
                nc.scalar.activation(i_act[:], ps[("i", mm_)][:], Sig,
                                     bias=bias_sb[:, mm_, 0:1], scale=INV_S)
                f_act = tpool.tile([p, b], f32, tag="f_act")
                nc.scalar.activation(f_act[:], ps[("f", mm_)][:], Sig,
                                     bias=bias_sb[:, mm_, 1:2], scale=INV_S)
                g_act = tpool.tile([p, b], f32, tag="g_act")
                nc.scalar.activation(g_act[:], ps[("g", mm_)][:], Tanh,
                                     bias=bias_sb[:, mm_, 2:3], scale=INV_S)

                c0_t = cpool.tile([p, b], f32, tag="c0")
                nc.gpsimd.dma_start(c0_t[:], c0T[:, mm_, :])

                t1 = tpool.tile([p, b], f32, tag="t1")
                nc.vector.tensor_mul(t1[:], f_act[:], c0_t[:])
                nc.vector.tensor_mul(i_act[:], i_act[:], g_act[:])
                c1_m = c1f_sb[:, mm_, :]
                nc.vector.tensor_add(c1_m, t1[:], i_act[:])
                if NEED_C18:
                    nc.scalar.activation(c18_sb[:, mm_, :], c1_m, Copy,
                                         scale=SA)
                if NEED_C116:
                    nc.vector.tensor_scalar_mul(c116_sb[:, mm_, :], c1_m, SA)
                nc.sync.dma_start(c1T[:, mm_, :], c1_m)

        # ---- phase 2: o gate + h1, in m-tile pairs ----
        P2_CONTRIBS = [
            ("o", "io", x16, x8),
            ("o", "ho", h16, h8),
            ("o", "co", c116_sb, c18_sb),
        ]
        for pair in [(m,) for m in range(mt)]:
            wts = {}
            for mm_ in pair:
                if mm_ in p2_pre:
                    (wts[("io", mm_)], wts[("ho", mm_)],
                     wts[("co", mm_)]) = p2_pre[mm_]
                else:
                    eng2 = nc.gpsimd if mm_ % 2 else None
                    wts[("io", mm_)] = load_w("io", "u0", mm_, eng=eng2)
                    wts[("ho", mm_)] = load_w("ho", "u1", mm_, eng=eng2)
                    wts[("co", mm_)] = load_w("co", "u2", mm_, eng=eng2)

            ps = {}
            for mm_ in pair:
                ps[("o", mm_)] = ppool.tile(
                    [p, b], f32, tag="ps", name=f"ps_o{mm_}")
            emit_pair(pair, P2_CONTRIBS, wts, ps)

            for mm_ in pair:
                o_act = tpool.tile([p, b], f32, tag="o_act")
                nc.scalar.activation(o_act[:], ps[("o", mm_)][:], Sig,
                                     bias=bias_sb[:, mm_, 3:4], scale=INV_S)
                tc1 = tpool.tile([p, b], f32, tag="tc1")
                nc.scalar.activation(tc1[:], c1f_sb[:, mm_, :], Tanh)
                h1_t = tpool.tile([p, b], f32, tag="h1")
                nc.vector.tensor_mul(h1_t[:], o_act[:], tc1[:])

                nc.sync.dma_start(ogT[:, mm_, :], o_act[:])
                nc.sync.dma_start(h1T[:, mm_, :], h1_t[:])

    nc.compile()
    return nc


_NC = None


def _get_nc():
    global _NC
    if _NC is None:
        _NC = _build(P, NK, MT, B)
    return _NC


# ---------------- host-side packing ----------------

def _pack_actT(a, dtype, scale=1.0):
    """(b, d) -> (128, d//128, b) with [ki, ko, b] = a[b, ko*128+ki]."""
    b, d = a.shape
    at = np.ascontiguousarray(a.T.reshape(d // P, P, b).transpose(1, 0, 2))
    if scale != 1.0:
        at = np.clip(at * scale, -240.0, 240.0)
    return at.astype(dtype, copy=False)


def _pack_w(W, dtype, kt_lo, kt_hi, scale):
    """(H, K) -> (H//128, 128, kt, 128) with [mt, ki, ko, m] = W[mt*128+m, ko*128+ki],
    keeping only k-tiles [kt_lo, kt_hi)."""
    H, K = W.shape
    r = (W * scale).reshape(H // P, P, K // P, P).transpose(0, 3, 2, 1)
    return np.ascontiguousarray(r[:, :, kt_lo:kt_hi]).astype(dtype)


def _unpack_out(o):
    """(128, mt, b) [p, m, b] -> (b, mt*128)."""
    p, m, b = o.shape
    return np.ascontiguousarray(o.transpose(2, 1, 0).reshape(b, m * p))


def kernel(x, h0, c0,
           W_ii, b_ii, W_hi, b_hi, W_if_, b_if_, W_hf, b_hf, W_cf, b_cf,
           W_ic, b_ic, W_hc, b_hc, W_io, b_io, W_ho, b_ho, W_co, b_co,
           _trace=False):
    from concourse.bass_utils import run_bass_kernel_spmd

    nc = _get_nc()

    x = np.asarray(x, dtype=np.float32)
    h0 = np.asarray(h0, dtype=np.float32)
    c0 = np.asarray(c0, dtype=np.float32)
    Ws = {n: np.asarray(a, dtype=np.float32)
          for n, a in zip(W_NAMES, (W_ii, W_hi, W_if_, W_hf, W_cf,
                                    W_ic, W_hc, W_io, W_ho, W_co))}
    (b_ii, b_hi, b_if_, b_hf, b_cf, b_ic, b_hc, b_io, b_ho, b_co) = [
        np.asarray(a, dtype=np.float32)
        for a in (b_ii, b_hi, b_if_, b_hf, b_cf, b_ic, b_hc, b_io, b_ho, b_co)
    ]

    # combined per-gate biases, packed [p, mt, gate]
    bias = np.stack(
        [
            (b_ii + b_hi).reshape(MT, P).T,
            (b_if_ + b_hf + b_cf).reshape(MT, P).T,
            (b_ic + b_hc).reshape(MT, P).T,
            (b_io + b_ho + b_co).reshape(MT, P).T,
        ],
        axis=2,
    ).astype(np.float32)

    w_packed = {}
    for n, W in Ws.items():
        n8 = CONFIG[n]
        nb = NK - n8
        if n8 > 0:
            w_packed[f"w8_{n}"] = _pack_w(W, F8, nb, NK, SW)
        if nb > 0:
            w_packed[f"w16_{n}"] = _pack_w(W, BF, 0, nb, SW)

    in_maps = []
    for core in range(N_CORES):
        s = slice(core * B, (core + 1) * B)
        m = {
            "c0T": _pack_actT(c0[s], np.float32),
            "bias": bias,
        }
        if NEED_X8:
            m["xT8"] = _pack_actT(x[s], F8, SA)
        if NEED_X16:
            m["xT16"] = _pack_actT(x[s], BF, SA)
        if NEED_H8:
            m["hT8"] = _pack_actT(h0[s], F8, SA)
        if NEED_H16:
            m["hT16"] = _pack_actT(h0[s], BF, SA)
        if NEED_C8:
            m["cT8"] = _pack_actT(c0[s], F8, SA)
        if NEED_C16:
            m["cT16"] = _pack_actT(c0[s], BF, SA)
        m.update(w_packed)
        in_maps.append(m)

    res = run_bass_kernel_spmd(nc, in_maps, list(range(N_CORES)), trace=_trace)

    o_g = np.empty((BATCH, HID), np.float32)
    h1 = np.empty((BATCH, HID), np.float32)
    c1 = np.empty((BATCH, HID), np.float32)
    for core in range(N_CORES):
        s = slice(core * B, (core + 1) * B)
        o_g[s] = _unpack_out(res.results[core]["ogT"])
        h1[s] = _unpack_out(res.results[core]["h1T"])
        c1[s] = _unpack_out(res.results[core]["c1T"])
    out = (o_g, h1, c1)
    if _trace:
        return out, res
    return out


# revision 33
# speedup vs baseline: 1.0204x; 1.0204x over previous
"""Trainium2 Bass kernel for the nn_LSTMCell problem.

Strategy: data-parallel over the batch dim (4096 -> 8 cores x 512), weights
replicated. All on-chip compute happens in "transposed" orientation
(hidden on PSUM partitions, batch on the free dim) so every matmul operand
can be DMA'd in its natural, contiguous layout:

    gate.T[h, b] = sum_k W.T[k, h] * act.T[k, b]
    matmul(out[M=h128, N=b512], lhsT=WT_tile[K=k128, M=h128], rhs=actT[K=k128, N=b512])

Mixed precision: most gate matmuls run in fp8(e4m3) with DoubleRow perf mode
(2 k-tiles per matmul, ~2x PE throughput); the error-critical matrices (the
tanh'd cell-candidate gate, and optionally parts of the output gate) stay in
bf16. All operands are pre-scaled on the host (W*256, act*16) so fp8 values
sit in the normal range; the 2^-12 descale is folded into the gate activation
instruction. PSUM accumulation is fp32 throughout, as is all elementwise math.

Per matrix, CONFIG[name] = number of 128-wide k-tiles (out of 16) computed in
fp8-DoubleRow; the first 16-n8 k-tiles run in bf16. Both forms accumulate
into the same PSUM (uniform operand scaling makes that legal).

Per core:
  phase 1: for each of 16 h-tiles: i/f/g gate matmuls, sigmoid/tanh,
           c1 = f*c0 + i*tanh(g)  -> c1 (fp32, kept in SBUF + DMA'd out),
           c1 scaled+cast to fp8 (matmul operand for the o gate).
  phase 2: for each of 16 h-tiles: o gate matmuls (incl. W_co @ c1.T),
           o = sigmoid(...), h1 = o * tanh(c1), DMA out.
"""

import numpy as np
import ml_dtypes
from contextlib import ExitStack

BF = ml_dtypes.bfloat16
F8 = ml_dtypes.float8_e4m3

N_CORES = 8
P = 128          # partition dim / k-tile size / m-tile size
BATCH = 4096
IN_DIM = 2048
HID = 2048
B = BATCH // N_CORES          # 512, batch per core = matmul free dim
NK = 2048 // P                # 16, k-tiles per weight matrix contraction
MT = HID // P                 # 16, output h-tiles

W_NAMES = ["ii", "hi", "if_", "hf", "cf", "ic", "hc", "io", "ho", "co"]

# k-tiles (of 16) per matrix computed in fp8-DoubleRow; rest in bf16.
# The g gate (ic/hc, goes through tanh into c1) dominates the fp8 error and
# stays bf16; io/ho are the next-largest contributors.
CONFIG = {
    "ii": 16, "hi": 16,
    "if_": 16, "hf": 16, "cf": 16,
    "ic": 0, "hc": 0,
    "io": 16, "ho": 16, "co": 16,
}

SW = 256.0   # host-side weight scale (all matrices, both dtypes)
SA = 16.0    # host-side activation scale (x/h/c0 and on-device c1)
INV_S = 1.0 / (SW * SA)

# which activation operand forms are needed on device
_X_MATS = ("ii", "if_", "ic", "io")
_H_MATS = ("hi", "hf", "hc", "ho")
NEED_X8 = any(CONFIG[n] > 0 for n in _X_MATS)
NEED_X16 = any(CONFIG[n] < NK for n in _X_MATS)
NEED_H8 = any(CONFIG[n] > 0 for n in _H_MATS)
NEED_H16 = any(CONFIG[n] < NK for n in _H_MATS)
NEED_C8 = CONFIG["cf"] > 0
NEED_C16 = CONFIG["cf"] < NK
NEED_C18 = CONFIG["co"] > 0
NEED_C116 = CONFIG["co"] < NK


def _build(p, nk, mt, b):
    import concourse.tile as tile
    from concourse import bacc, mybir

    bf16, f32 = mybir.dt.bfloat16, mybir.dt.float32
    f8 = mybir.dt.float8e4
    Sig = mybir.ActivationFunctionType.Sigmoid
    Tanh = mybir.ActivationFunctionType.Tanh
    Copy = mybir.ActivationFunctionType.Copy
    DR = mybir.MatmulPerfMode.DoubleRow

    nc = bacc.Bacc(
        "TRN2",
        target_bir_lowering=False,
        debug=False,
        num_devices=N_CORES,
    )

    def act_in(name, dt):
        return nc.dram_tensor(name, [p, nk, b], dt, kind="ExternalInput").ap()

    xT8 = act_in("xT8", f8) if NEED_X8 else None
    xT16 = act_in("xT16", bf16) if NEED_X16 else None
    hT8 = act_in("hT8", f8) if NEED_H8 else None
    hT16 = act_in("hT16", bf16) if NEED_H16 else None
    cT8 = act_in("cT8", f8) if NEED_C8 else None
    cT16 = act_in("cT16", bf16) if NEED_C16 else None
    c0T = nc.dram_tensor("c0T", [p, mt, b], f32, kind="ExternalInput").ap()
    bias = nc.dram_tensor("bias", [p, mt, 4], f32, kind="ExternalInput").ap()

    w8, w16 = {}, {}
    for n in W_NAMES:
        n8 = CONFIG[n]
        if n8 > 0:
            w8[n] = nc.dram_tensor(
                f"w8_{n}", [mt, p, n8, p], f8, kind="ExternalInput").ap()
        if n8 < nk:
            w16[n] = nc.dram_tensor(
                f"w16_{n}", [mt, p, nk - n8, p], bf16, kind="ExternalInput").ap()

    ogT = nc.dram_tensor("ogT", [p, mt, b], f32, kind="ExternalOutput").ap()
    h1T = nc.dram_tensor("h1T", [p, mt, b], f32, kind="ExternalOutput").ap()
    c1T = nc.dram_tensor("c1T", [p, mt, b], f32, kind="ExternalOutput").ap()

    with tile.TileContext(nc) as tc, ExitStack() as ctx:
        acts = ctx.enter_context(tc.tile_pool(name="acts", bufs=1))
        wpool = ctx.enter_context(tc.tile_pool(name="w", bufs=3))
        cpool = ctx.enter_context(tc.tile_pool(name="c0", bufs=2))
        tpool = ctx.enter_context(tc.tile_pool(name="temps", bufs=2))
        ppool = ctx.enter_context(tc.tile_pool(name="psum", bufs=8, space="PSUM"))

        # resident activation tensors. Loads go on the gpsimd/sync DMA issue
        # queues, split into chunks so the first matmuls — which only need the
        # first x chunks plus one weight slab — start early.
        CH = 4  # k-tiles per DMA chunk
        sb = {}
        loads = []
        # spread the activation preload over four DMA issue queues so the
        # early m-tiles (whose matmuls consume data as fast as it lands) are
        # not bottlenecked on a single queue's descriptor rate. x8 gets a
        # small leading chunk so the very first matmul can start early.
        for key, need, src, dt, eng, chunks in (
            ("x8", NEED_X8, xT8, f8, nc.gpsimd, (2, 2, 4, 4, 4)),
            ("x16", NEED_X16, xT16, bf16, nc.sync, (2, 2, 4, 4, 4)),
            ("h8", NEED_H8, hT8, f8, nc.gpsimd, (4, 4, 4, 4)),
            ("h16", NEED_H16, hT16, bf16, nc.sync, (4, 4, 4, 4)),
            ("c8", NEED_C8, cT8, f8, nc.gpsimd, (8, 8)),
            ("c16", NEED_C16, cT16, bf16, nc.sync, (8, 8)),
        ):
            if need:
                sb[key] = acts.tile([p, nk, b], dt, tag=key, name=key + "_sb")
                loads.append((src, sb[key], eng, chunks))
        bias_sb = acts.tile([p, mt, 4], f32, tag="bias")
        nc.scalar.dma_start(bias_sb[:], bias[:])
        for src, dst, eng, chunks in loads:
            c = 0
            for ci, ch in enumerate(chunks):
                # the first x8 chunks gate the very first matmul: issue them
                # on the hardware-DGE scalar queue ahead of the weight slabs
                e = nc.scalar if (dst is sb.get("x8") and ci < 2) else eng
                e.dma_start(dst[:, c:c + ch, :], src[:, c:c + ch, :])
                c += ch
        c1f_sb = acts.tile([p, mt, b], f32, tag="c1f")    # new cell state, fp32
        c18_sb = (acts.tile([p, mt, b], f8, tag="c18", name="c18_sb")
                  if NEED_C18 else None)
        c116_sb = (acts.tile([p, mt, b], bf16, tag="c116", name="c116_sb")
                   if NEED_C116 else None)

        def load_w(name, tag, m, chunks=1, eng=None):
            """Load this matrix's bf16 part and fp8 part; returns (t16, t8)."""
            t16 = t8 = None
            n8 = CONFIG[name]
            if n8 < nk:
                nkp = nk - n8
                t16 = wpool.tile([p, nkp, p], bf16, tag=tag + "b")
                step = max(1, nkp // chunks)
                for c in range(0, nkp, step):
                    (eng or nc.sync).dma_start(
                        t16[:, c:c + step], w16[name][m, :, c:c + step])
            if n8 > 0:
                t8 = wpool.tile([p, n8, p], f8, tag=tag + "a")
                step = max(1, n8 // chunks)
                for c in range(0, n8, step):
                    (eng or nc.sync).dma_start(
                        t8[:, c:c + step], w8[name][m, :, c:c + step])
            return t16, t8

        def emit_pair(pair, contribs, wts, ps):
            """Emit all matmuls for a pair of m-tiles, k-major with the pair
            innermost, so each activation k-chunk feeds both m-tiles' matmuls
            and the DMA demand rate per chunk is halved (matters during the
            ramp, where the first m-tiles otherwise outrun the preload).

            contribs: ordered list of (gate, matrix-name, act16, act8);
            wts[(name, m)] = (t16, t8); ps[(gate, m)] = psum tile.
            """
            tot = {}
            for gate, name, _, _ in contribs:
                n8 = CONFIG[name]
                tot[gate] = tot.get(gate, 0) + (nk - n8) + n8 // 2
            idx = {g: 0 for g in tot}
            for gate, name, a16, a8 in contribs:
                n8 = CONFIG[name]
                nb = nk - n8
                for ko in range(nb):
                    st = idx[gate] == 0
                    sp = idx[gate] == tot[gate] - 1
                    for mm_ in pair:
                        nc.tensor.matmul(
                            ps[(gate, mm_)][:], lhsT=wts[(name, mm_)][0][:, ko],
                            rhs=a16[:, ko], start=st, stop=sp,
                        )
                    idx[gate] += 1
                for ko2 in range(0, n8, 2):
                    st = idx[gate] == 0
                    sp = idx[gate] == tot[gate] - 1
                    for mm_ in pair:
                        nc.tensor.matmul(
                            ps[(gate, mm_)][:],
                            lhsT=wts[(name, mm_)][1][:, ko2:ko2 + 2],
                            rhs=a8[:, nb + ko2:nb + ko2 + 2],
                            start=st, stop=sp, perf_mode=DR,
                        )
                    idx[gate] += 1

        x16, x8 = sb.get("x16"), sb.get("x8")
        h16, h8 = sb.get("h16"), sb.get("h8")

        # ---- phase 1: i/f/g gates + new cell state, in m-tile pairs ----
        # x-term weights load (and matmul) first so the first m-tile's PE work
        # starts as soon as x chunks land, while h/c still stream in.
        P1_CONTRIBS = [
            ("i", "ii", x16, x8),
            ("f", "if_", x16, x8),
            ("g", "ic", x16, x8),
            ("i", "hi", h16, h8),
            ("f", "hf", h16, h8),
            ("g", "hc", h16, h8),
            ("f", "cf", sb.get("c16"), sb.get("c8")),
        ]
        # All m-tiles run as singletons: k-major pairing halves the
        # activation demand rate on paper, but measured consistently slower —
        # batching a pair's weight-slab issues trips the per-queue
        # outstanding-DMA limit and serializes the ramp transfers.
        p1_groups = [(m,) for m in range(mt)]
        p2_pre = {}
        for pair in p1_groups:
            wts = {}
            for mm_ in pair:
                # the first m-tiles' slab issues go on the otherwise-idle
                # scalar engine queue: the sync/gpsimd queues are saturated
                # streaming the activation preload during the ramp.
                if mm_ == 0:
                    w_eng, ch = nc.scalar, 2
                elif mm_ in (1, 2):
                    w_eng, ch = nc.scalar, 1
                else:
                    w_eng, ch = None, 1
                for name, tag in (("ii", "w0"), ("if_", "w2"), ("ic", "w5"),
                                  ("hi", "w1"), ("hf", "w3"), ("hc", "w6"),
                                  ("cf", "w4")):
                    wts[(name, mm_)] = load_w(name, tag, mm_,
                                              chunks=ch, eng=w_eng)
                if mm_ == mt - 2:
                    # prefetch the first two phase-2 weight slab sets on the
                    # scalar queue so phase 2 starts without a DMA stall
                    for pm in (0, 1):
                        p2_pre[pm] = (load_w("io", "u0", pm, eng=nc.scalar),
                                      load_w("ho", "u1", pm, eng=nc.scalar),
                                      load_w("co", "u2", pm, eng=nc.scalar))

            ps = {}
            for gate in ("i", "f", "g"):
                for mm_ in pair:
                    ps[(gate, mm_)] = ppool.tile(
                        [p, b], f32, tag="ps", name=f"ps_{gate}{mm_}")
            emit_pair(pair, P1_CONTRIBS, wts, ps)

            for mm_ in pair:
                i_act = tpool.tile([p, b], f32, tag="i_act")
                nc.scalar.activation(i_act[:], ps[("i", mm_)][:], Sig,
                                     bias=bias_sb[:, mm_, 0:1], scale=INV_S)
                f_act = tpool.tile([p, b], f32, tag="f_act")
                nc.scalar.activation(f_act[:], ps[("f", mm_)][:], Sig,
                                     bias=bias_sb[:, mm_, 1:2], scale=INV_S)
                g_act = tpool.tile([p, b], f32, tag="g_act")
                nc.scalar.activation(g_act[:], ps[("g", mm_)][:], Tanh,
                                     bias=bias_sb[:, mm_, 2:3], scale=INV_S)

                c0_t = cpool.tile([p, b], f32, tag="c0")
                nc.gpsimd.dma_start(c0_t[:], c0T[:, mm_, :])

                t1 = tpool.tile([p, b], f32, tag="t1")
                nc.vector.tensor_mul(t1[:], f_act[:], c0_t[:])
                nc.vector.tensor_mul(i_act[:], i_act[:], g_act[:])
                c1_m = c1f_sb[:, mm_, :]
                nc.vector.tensor_add(c1_m, t1[:], i_act[:])
                if NEED_C18:
                    nc.scalar.activation(c18_sb[:, mm_, :], c1_m, Copy,
                                         scale=SA)
                if NEED_C116:
                    nc.vector.tensor_scalar_mul(c116_sb[:, mm_, :], c1_m, SA)
                nc.sync.dma_start(c1T[:, mm_, :], c1_m)

        # ---- phase 2: o gate + h1, in m-tile pairs ----
        P2_CONTRIBS = [
            ("o", "io", x16, x8),
            ("o", "ho", h16, h8),
            ("o", "co", c116_sb, c18_sb),
        ]
        for pair in [(m,) for m in range(mt)]:
            wts = {}
            for mm_ in pair:
                if mm_ in p2_pre:
                    (wts[("io", mm_)], wts[("ho", mm_)],
                     wts[("co", mm_)]) = p2_pre[mm_]
                else:
                    eng2 = nc.gpsimd if mm_ % 2 else None
                    wts[("io", mm_)] = load_w("io", "u0", mm_, eng=eng2)
                    wts[("ho", mm_)] = load_w("ho", "u1", mm_, eng=eng2)
                    wts[("co", mm_)] = load_w("co", "u2", mm_, eng=eng2)

            ps = {}
            for mm_ in pair:
                ps[("o", mm_)] = ppool.tile(
                    [p, b], f32, tag="ps", name=f"ps_o{mm_}")
            emit_pair(pair, P2_CONTRIBS, wts, ps)

            for mm_ in pair:
                o_act = tpool.tile([p, b], f32, tag="o_act")
                nc.scalar.activation(o_act[:], ps[("o", mm_)][:], Sig,
                                     bias=bias_sb[:, mm_, 3:4], scale=INV_S)
                tc1 = tpool.tile([p, b], f32, tag="tc1")
                nc.scalar.activation(tc1[:], c1f_sb[:, mm_, :], Tanh)
                h1_t = tpool.tile([p, b], f32, tag="h1")
                nc.vector.tensor_mul(h1_t[:], o_act[:], tc1[:])

                nc.sync.dma_start(ogT[:, mm_, :], o_act[:])
                nc.sync.dma_start(h1T[:, mm_, :], h1_t[:])

    nc.compile()
    return nc


_NC = None


def _get_nc():
    global _NC
    if _NC is None:
        _NC = _build(P, NK, MT, B)
    return _NC


# ---------------- host-side packing ----------------

def _pack_actT(a, dtype, scale=1.0):
    """(b, d) -> (128, d//128, b) with [ki, ko, b] = a[b, ko*128+ki]."""
    b, d = a.shape
    at = np.ascontiguousarray(a.T.reshape(d // P, P, b).transpose(1, 0, 2))
    if scale != 1.0:
        at = np.clip(at * scale, -240.0, 240.0)
    return at.astype(dtype, copy=False)


def _pack_w(W, dtype, kt_lo, kt_hi, scale):
    """(H, K) -> (H//128, 128, kt, 128) with [mt, ki, ko, m] = W[mt*128+m, ko*128+ki],
    keeping only k-tiles [kt_lo, kt_hi)."""
    H, K = W.shape
    r = (W * scale).reshape(H // P, P, K // P, P).transpose(0, 3, 2, 1)
    return np.ascontiguousarray(r[:, :, kt_lo:kt_hi]).astype(dtype)


def _unpack_out(o):
    """(128, mt, b) [p, m, b] -> (b, mt*128)."""
    p, m, b = o.shape
    return np.ascontiguousarray(o.transpose(2, 1, 0).reshape(b, m * p))


def kernel(x, h0, c0,
           W_ii, b_ii, W_hi, b_hi, W_if_, b_if_, W_hf, b_hf, W_cf, b_cf,
           W_ic, b_ic, W_hc, b_hc, W_io, b_io, W_ho, b_ho, W_co, b_co,
           _trace=False):
    from concourse.bass_utils import run_bass_kernel_spmd

    nc = _get_nc()

    x = np.asarray(x, dtype=np.float32)
    h0 = np.asarray(h0, dtype=np.float32)
    c0 = np.asarray(c0, dtype=np.float32)
    Ws = {n: np.asarray(a, dtype=np.float32)
          for n, a in zip(W_NAMES, (W_ii, W_hi, W_if_, W_hf, W_cf,
                                    W_ic, W_hc, W_io, W_ho, W_co))}
    (b_ii, b_hi, b_if_, b_hf, b_cf, b_ic, b_hc, b_io, b_ho, b_co) = [
        np.asarray(a, dtype=np.float32)
        for a in (b_ii, b_hi, b_if_, b_hf, b_cf, b_ic, b_hc, b_io, b_ho, b_co)
    ]

    # combined per-gate biases, packed [p, mt, gate]
    bias = np.stack(
        [
            (b_ii + b_hi).reshape(MT, P).T,
            (b_if_ + b_hf + b_cf).reshape(MT, P).T,
            (b_ic + b_hc).reshape(MT, P).T,
            (b_io + b_ho + b_co).reshape(MT, P).T,
        ],
        axis=2,
    ).astype(np.float32)

    w_packed = {}
    for n, W in Ws.items():
        n8 = CONFIG[n]
        nb = NK - n8
        if n8 > 0:
            w_packed[f"w8_{n}"] = _pack_w(W, F8, nb, NK, SW)
        if nb > 0:
            w_packed[f"w16_{n}"] = _pack_w(W, BF, 0, nb, SW)

    in_maps = []
    for core in range(N_CORES):
        s = slice(core * B, (core + 1) * B)
        m = {
            "c0T": _pack_actT(c0[s], np.float32),
            "bias": bias,
        }
        if NEED_X8:
            m["xT8"] = _pack_actT(x[s], F8, SA)
        if NEED_X16:
            m["xT16"] = _pack_actT(x[s], BF, SA)
        if NEED_H8:
            m["hT8"] = _pack_actT(h0[s], F8, SA)
        if NEED_H16:
            m["hT16"] = _pack_actT(h0[s], BF, SA)
        if NEED_C8:
            m["cT8"] = _pack_actT(c0[s], F8, SA)
        if NEED_C16:
            m["cT16"] = _pack_actT(c0[s], BF, SA)
        m.update(w_packed)
        in_maps.append(m)

    res = run_bass_kernel_spmd(nc, in_maps, list(range(N_CORES)), trace=_trace)

    o_g = np.empty((BATCH, HID), np.float32)
    h1 = np.empty((BATCH, HID), np.float32)
    c1 = np.empty((BATCH, HID), np.float32)
    for core in range(N_CORES):
        s = slice(core * B, (core + 1) * B)
        o_g[s] = _unpack_out(res.results[core]["ogT"])
        h1[s] = _unpack_out(res.results[core]["h1T"])
        c1[s] = _unpack_out(res.results[core]["c1T"])
    out = (o_g, h1, c1)
    if _trace:
        return out, res
    return out


# revision 37
# speedup vs baseline: 1.0244x; 1.0039x over previous
"""Trainium2 Bass kernel for the nn_LSTMCell problem.

Strategy: data-parallel over the batch dim (4096 -> 8 cores x 512), weights
replicated. All on-chip compute happens in "transposed" orientation
(hidden on PSUM partitions, batch on the free dim) so every matmul operand
can be DMA'd in its natural, contiguous layout:

    gate.T[h, b] = sum_k W.T[k, h] * act.T[k, b]
    matmul(out[M=h128, N=b512], lhsT=WT_tile[K=k128, M=h128], rhs=actT[K=k128, N=b512])

Mixed precision: most gate matmuls run in fp8(e4m3) with DoubleRow perf mode
(2 k-tiles per matmul, ~2x PE throughput); the error-critical matrices (the
tanh'd cell-candidate gate, and optionally parts of the output gate) stay in
bf16. All operands are pre-scaled on the host (W*256, act*16) so fp8 values
sit in the normal range; the 2^-12 descale is folded into the gate activation
instruction. PSUM accumulation is fp32 throughout, as is all elementwise math.

Per matrix, CONFIG[name] = number of 128-wide k-tiles (out of 16) computed in
fp8-DoubleRow; the first 16-n8 k-tiles run in bf16. Both forms accumulate
into the same PSUM (uniform operand scaling makes that legal).

Per core:
  phase 1: for each of 16 h-tiles: i/f/g gate matmuls, sigmoid/tanh,
           c1 = f*c0 + i*tanh(g)  -> c1 (fp32, kept in SBUF + DMA'd out),
           c1 scaled+cast to fp8 (matmul operand for the o gate).
  phase 2: for each of 16 h-tiles: o gate matmuls (incl. W_co @ c1.T),
           o = sigmoid(...), h1 = o * tanh(c1), DMA out.
"""

import numpy as np
import ml_dtypes
from contextlib import ExitStack

BF = ml_dtypes.bfloat16
F8 = ml_dtypes.float8_e4m3

N_CORES = 8
P = 128          # partition dim / k-tile size / m-tile size
BATCH = 4096
IN_DIM = 2048
HID = 2048
B = BATCH // N_CORES          # 512, batch per core = matmul free dim
NK = 2048 // P                # 16, k-tiles per weight matrix contraction
MT = HID // P                 # 16, output h-tiles

W_NAMES = ["ii", "hi", "if_", "hf", "cf", "ic", "hc", "io", "ho", "co"]

# k-tiles (of 16) per matrix computed in fp8-DoubleRow; rest in bf16.
# The g gate (ic/hc, goes through tanh into c1) dominates the fp8 error and
# stays bf16; io/ho are the next-largest contributors.
CONFIG = {
    "ii": 16, "hi": 16,
    "if_": 16, "hf": 16, "cf": 16,
    "ic": 0, "hc": 0,
    "io": 16, "ho": 16, "co": 16,
}

SW = 256.0   # host-side weight scale (all matrices, both dtypes)
SA = 16.0    # host-side activation scale (x/h/c0 and on-device c1)
INV_S = 1.0 / (SW * SA)

# which activation operand forms are needed on device
_X_MATS = ("ii", "if_", "ic", "io")
_H_MATS = ("hi", "hf", "hc", "ho")
NEED_X8 = any(CONFIG[n] > 0 for n in _X_MATS)
NEED_X16 = any(CONFIG[n] < NK for n in _X_MATS)
NEED_H8 = any(CONFIG[n] > 0 for n in _H_MATS)
NEED_H16 = any(CONFIG[n] < NK for n in _H_MATS)
NEED_C8 = CONFIG["cf"] > 0
NEED_C16 = CONFIG["cf"] < NK
NEED_C18 = CONFIG["co"] > 0
NEED_C116 = CONFIG["co"] < NK
# fp8 forms derived on-device from the bf16 forms when both exist
DERIVE_X8 = NEED_X8 and NEED_X16
DERIVE_H8 = NEED_H8 and NEED_H16


def _build(p, nk, mt, b):
    import concourse.tile as tile
    from concourse import bacc, mybir

    bf16, f32 = mybir.dt.bfloat16, mybir.dt.float32
    f8 = mybir.dt.float8e4
    Sig = mybir.ActivationFunctionType.Sigmoid
    Tanh = mybir.ActivationFunctionType.Tanh
    Copy = mybir.ActivationFunctionType.Copy
    DR = mybir.MatmulPerfMode.DoubleRow

    nc = bacc.Bacc(
        "TRN2",
        target_bir_lowering=False,
        debug=False,
        num_devices=N_CORES,
    )

    def act_in(name, dt):
        return nc.dram_tensor(name, [p, nk, b], dt, kind="ExternalInput").ap()

    xT8 = act_in("xT8", f8) if NEED_X8 and not DERIVE_X8 else None
    xT16 = act_in("xT16", bf16) if NEED_X16 else None
    hT8 = act_in("hT8", f8) if NEED_H8 and not DERIVE_H8 else None
    hT16 = act_in("hT16", bf16) if NEED_H16 else None
    cT8 = act_in("cT8", f8) if NEED_C8 else None
    cT16 = act_in("cT16", bf16) if NEED_C16 else None
    c0T = nc.dram_tensor("c0T", [p, mt, b], f32, kind="ExternalInput").ap()
    bias = nc.dram_tensor("bias", [p, mt, 4], f32, kind="ExternalInput").ap()

    w8, w16 = {}, {}
    for n in W_NAMES:
        n8 = CONFIG[n]
        if n8 > 0:
            w8[n] = nc.dram_tensor(
                f"w8_{n}", [mt, p, n8, p], f8, kind="ExternalInput").ap()
        if n8 < nk:
            w16[n] = nc.dram_tensor(
                f"w16_{n}", [mt, p, nk - n8, p], bf16, kind="ExternalInput").ap()

    ogT = nc.dram_tensor("ogT", [p, mt, b], f32, kind="ExternalOutput").ap()
    h1T = nc.dram_tensor("h1T", [p, mt, b], f32, kind="ExternalOutput").ap()
    c1T = nc.dram_tensor("c1T", [p, mt, b], f32, kind="ExternalOutput").ap()

    with tile.TileContext(nc) as tc, ExitStack() as ctx:
        acts = ctx.enter_context(tc.tile_pool(name="acts", bufs=1))
        wpool = ctx.enter_context(tc.tile_pool(name="w", bufs=3))
        cpool = ctx.enter_context(tc.tile_pool(name="c0", bufs=2))
        tpool = ctx.enter_context(tc.tile_pool(name="temps", bufs=2))
        ppool = ctx.enter_context(tc.tile_pool(name="psum", bufs=8, space="PSUM"))

        # resident activation tensors. Loads go on the gpsimd/sync DMA issue
        # queues, split into chunks so the first matmuls — which only need the
        # first x chunks plus one weight slab — start early.
        CH = 4  # k-tiles per DMA chunk
        sb = {}
        loads = []
        # spread the activation preload over four DMA issue queues so the
        # early m-tiles (whose matmuls consume data as fast as it lands) are
        # not bottlenecked on a single queue's descriptor rate. x8 gets a
        # small leading chunk so the very first matmul can start early.
        # When both the bf16 and fp8 form of a tensor are needed, only the
        # bf16 form is DMA'd: the fp8 form is re-quantized on-device by the
        # otherwise-idle vector engine (the operands are pre-scaled on host,
        # so a dtype-converting copy is the whole cast). This cuts the
        # bandwidth-bound activation preload from 7MB to 5MB.
        derive = []
        for key, need, src, dt, eng, chunks in (
            ("x16", NEED_X16, xT16, bf16, nc.sync, (2, 2, 4, 4, 4)),
            ("x8", NEED_X8, xT8, f8, nc.gpsimd, (2, 2, 4, 4, 4)),
            ("h16", NEED_H16, hT16, bf16, nc.sync, (4, 4, 4, 4)),
            ("h8", NEED_H8, hT8, f8, nc.gpsimd, (4, 4, 4, 4)),
            ("c8", NEED_C8, cT8, f8, nc.gpsimd, (8, 8)),
            ("c16", NEED_C16, cT16, bf16, nc.sync, (8, 8)),
        ):
            if need:
                sb[key] = acts.tile([p, nk, b], dt, tag=key, name=key + "_sb")
                src16 = {"x8": "x16", "h8": "h16"}.get(key)
                if src16 and src16 in sb:
                    derive.append((key, src16, chunks))
                else:
                    loads.append((src, sb[key], eng, chunks))
        bias_sb = acts.tile([p, mt, 4], f32, tag="bias")
        nc.scalar.dma_start(bias_sb[:], bias[:])
        for src, dst, eng, chunks in loads:
            c = 0
            for ch in chunks:
                eng.dma_start(dst[:, c:c + ch, :], src[:, c:c + ch, :])
                c += ch
        for key, src16, chunks in derive:
            c = 0
            for ch in chunks:
                nc.vector.tensor_copy(out=sb[key][:, c:c + ch, :],
                                      in_=sb[src16][:, c:c + ch, :])
                c += ch
        c1f_sb = acts.tile([p, mt, b], f32, tag="c1f")    # new cell state, fp32
        c18_sb = (acts.tile([p, mt, b], f8, tag="c18", name="c18_sb")
                  if NEED_C18 else None)
        c116_sb = (acts.tile([p, mt, b], bf16, tag="c116", name="c116_sb")
                   if NEED_C116 else None)

        def load_w(name, tag, m, chunks=1, eng=None):
            """Load this matrix's bf16 part and fp8 part; returns (t16, t8)."""
            t16 = t8 = None
            n8 = CONFIG[name]
            if n8 < nk:
                nkp = nk - n8
                t16 = wpool.tile([p, nkp, p], bf16, tag=tag + "b")
                step = max(1, nkp // chunks)
                for c in range(0, nkp, step):
                    (eng or nc.sync).dma_start(
                        t16[:, c:c + step], w16[name][m, :, c:c + step])
            if n8 > 0:
                t8 = wpool.tile([p, n8, p], f8, tag=tag + "a")
                step = max(1, n8 // chunks)
                for c in range(0, n8, step):
                    (eng or nc.sync).dma_start(
                        t8[:, c:c + step], w8[name][m, :, c:c + step])
            return t16, t8

        def emit_pair(pair, contribs, wts, ps):
            """Emit all matmuls for a pair of m-tiles, k-major with the pair
            innermost, so each activation k-chunk feeds both m-tiles' matmuls
            and the DMA demand rate per chunk is halved (matters during the
            ramp, where the first m-tiles otherwise outrun the preload).

            contribs: ordered list of (gate, matrix-name, act16, act8);
            wts[(name, m)] = (t16, t8); ps[(gate, m)] = psum tile.
            """
            tot = {}
            for gate, name, _, _ in contribs:
                n8 = CONFIG[name]
                tot[gate] = tot.get(gate, 0) + (nk - n8) + n8 // 2
            idx = {g: 0 for g in tot}
            for gate, name, a16, a8 in contribs:
                n8 = CONFIG[name]
                nb = nk - n8
                for ko in range(nb):
                    st = idx[gate] == 0
                    sp = idx[gate] == tot[gate] - 1
                    for mm_ in pair:
                        nc.tensor.matmul(
                            ps[(gate, mm_)][:], lhsT=wts[(name, mm_)][0][:, ko],
                            rhs=a16[:, ko], start=st, stop=sp,
                        )
                    idx[gate] += 1
                for ko2 in range(0, n8, 2):
                    st = idx[gate] == 0
                    sp = idx[gate] == tot[gate] - 1
                    for mm_ in pair:
                        nc.tensor.matmul(
                            ps[(gate, mm_)][:],
                            lhsT=wts[(name, mm_)][1][:, ko2:ko2 + 2],
                            rhs=a8[:, nb + ko2:nb + ko2 + 2],
                            start=st, stop=sp, perf_mode=DR,
                        )
                    idx[gate] += 1

        x16, x8 = sb.get("x16"), sb.get("x8")
        h16, h8 = sb.get("h16"), sb.get("h8")

        # ---- phase 1: i/f/g gates + new cell state, in m-tile pairs ----
        # x-term weights load (and matmul) first so the first m-tile's PE work
        # starts as soon as x chunks land, while h/c still stream in.
        P1_CONTRIBS = [
            ("i", "ii", x16, x8),
            ("f", "if_", x16, x8),
            ("g", "ic", x16, x8),
            ("i", "hi", h16, h8),
            ("f", "hf", h16, h8),
            ("g", "hc", h16, h8),
            ("f", "cf", sb.get("c16"), sb.get("c8")),
        ]
        # All m-tiles run as singletons: k-major pairing halves the
        # activation demand rate on paper, but measured consistently slower —
        # batching a pair's weight-slab issues trips the per-queue
        # outstanding-DMA limit and serializes the ramp transfers.
        p1_groups = [(m,) for m in range(mt)]
        p2_pre = {}
        for pair in p1_groups:
            wts = {}
            for mm_ in pair:
                # the first m-tiles' slab issues go on the otherwise-idle
                # scalar engine queue: the sync/gpsimd queues are saturated
                # streaming the activation preload during the ramp.
                if mm_ == 0:
                    w_eng, ch = nc.scalar, 2
                elif mm_ in (1, 2):
                    w_eng, ch = nc.scalar, 1
                else:
                    w_eng, ch = None, 1
                for name, tag in (("ii", "w0"), ("if_", "w2"), ("ic", "w5"),
                                  ("hi", "w1"), ("hf", "w3"), ("hc", "w6"),
                                  ("cf", "w4")):
                    wts[(name, mm_)] = load_w(name, tag, mm_,
                                              chunks=ch, eng=w_eng)
                if mm_ == mt - 2:
                    # prefetch the first two phase-2 weight slab sets on the
                    # scalar queue so phase 2 starts without a DMA stall
                    for pm in (0, 1):
                        p2_pre[pm] = (load_w("io", "u0", pm, eng=nc.scalar),
                                      load_w("ho", "u1", pm, eng=nc.scalar),
                                      load_w("co", "u2", pm, eng=nc.scalar))

            ps = {}
            for gate in ("i", "f", "g"):
                for mm_ in pair:
                    ps[(gate, mm_)] = ppool.tile(
                        [p, b], f32, tag="ps", name=f"ps_{gate}{mm_}")
            emit_pair(pair, P1_CONTRIBS, wts, ps)

            for mm_ in pair:
                i_act = tpool.tile([p, b], f32, tag="i_act")
                nc.scalar.activation(i_act[:], ps[("i", mm_)][:], Sig,
                                     bias=bias_sb[:, mm_, 0:1], scale=INV_S)
                f_act = tpool.tile([p, b], f32, tag="f_act")
                nc.scalar.activation(f_act[:], ps[("f", mm_)][:], Sig,
                                     bias=bias_sb[:, mm_, 1:2], scale=INV_S)
                g_act = tpool.tile([p, b], f32, tag="g_act")
                nc.scalar.activation(g_act[:], ps[("g", mm_)][:], Tanh,
                                     bias=bias_sb[:, mm_, 2:3], scale=INV_S)

                c0_t = cpool.tile([p, b], f32, tag="c0")
                nc.gpsimd.dma_start(c0_t[:], c0T[:, mm_, :])

                t1 = tpool.tile([p, b], f32, tag="t1")
                nc.vector.tensor_mul(t1[:], f_act[:], c0_t[:])
                nc.vector.tensor_mul(i_act[:], i_act[:], g_act[:])
                c1_m = c1f_sb[:, mm_, :]
                nc.vector.tensor_add(c1_m, t1[:], i_act[:])
                if NEED_C18:
                    nc.scalar.activation(c18_sb[:, mm_, :], c1_m, Copy,
                                         scale=SA)
                if NEED_C116:
                    nc.vector.tensor_scalar_mul(c116_sb[:, mm_, :], c1_m, SA)
                nc.sync.dma_start(c1T[:, mm_, :], c1_m)

        # ---- phase 2: o gate + h1, in m-tile pairs ----
        P2_CONTRIBS = [
            ("o", "io", x16, x8),
            ("o", "ho", h16, h8),
            ("o", "co", c116_sb, c18_sb),
        ]
        for pair in [(m,) for m in range(mt)]:
            wts = {}
            for mm_ in pair:
                if mm_ in p2_pre:
                    (wts[("io", mm_)], wts[("ho", mm_)],
                     wts[("co", mm_)]) = p2_pre[mm_]
                else:
                    eng2 = nc.gpsimd if mm_ % 2 else None
                    wts[("io", mm_)] = load_w("io", "u0", mm_, eng=eng2)
                    wts[("ho", mm_)] = load_w("ho", "u1", mm_, eng=eng2)
                    wts[("co", mm_)] = load_w("co", "u2", mm_, eng=eng2)

            ps = {}
            for mm_ in pair:
                ps[("o", mm_)] = ppool.tile(
                    [p, b], f32, tag="ps", name=f"ps_o{mm_}")
            emit_pair(pair, P2_CONTRIBS, wts, ps)

            for mm_ in pair:
                o_act = tpool.tile([p, b], f32, tag="o_act")
                nc.scalar.activation(o_act[:], ps[("o", mm_)][:], Sig,
                                     bias=bias_sb[:, mm_, 3:4], scale=INV_S)
                tc1 = tpool.tile([p, b], f32, tag="tc1")
                nc.scalar.activation(tc1[:], c1f_sb[:, mm_, :], Tanh)
                h1_t = tpool.tile([p, b], f32, tag="h1")
                nc.vector.tensor_mul(h1_t[:], o_act[:], tc1[:])

                nc.sync.dma_start(ogT[:, mm_, :], o_act[:])
                nc.sync.dma_start(h1T[:, mm_, :], h1_t[:])

    nc.compile()
    return nc


_NC = None


def _get_nc():
    global _NC
    if _NC is None:
        _NC = _build(P, NK, MT, B)
    return _NC


# ---------------- host-side packing ----------------

def _pack_actT(a, dtype, scale=1.0):
    """(b, d) -> (128, d//128, b) with [ki, ko, b] = a[b, ko*128+ki]."""
    b, d = a.shape
    at = np.ascontiguousarray(a.T.reshape(d // P, P, b).transpose(1, 0, 2))
    if scale != 1.0:
        at = np.clip(at * scale, -240.0, 240.0)
    return at.astype(dtype, copy=False)


def _pack_w(W, dtype, kt_lo, kt_hi, scale):
    """(H, K) -> (H//128, 128, kt, 128) with [mt, ki, ko, m] = W[mt*128+m, ko*128+ki],
    keeping only k-tiles [kt_lo, kt_hi)."""
    H, K = W.shape
    r = (W * scale).reshape(H // P, P, K // P, P).transpose(0, 3, 2, 1)
    return np.ascontiguousarray(r[:, :, kt_lo:kt_hi]).astype(dtype)


def _unpack_out(o):
    """(128, mt, b) [p, m, b] -> (b, mt*128)."""
    p, m, b = o.shape
    return np.ascontiguousarray(o.transpose(2, 1, 0).reshape(b, m * p))


def kernel(x, h0, c0,
           W_ii, b_ii, W_hi, b_hi, W_if_, b_if_, W_hf, b_hf, W_cf, b_cf,
           W_ic, b_ic, W_hc, b_hc, W_io, b_io, W_ho, b_ho, W_co, b_co,
           _trace=False):
    from concourse.bass_utils import run_bass_kernel_spmd

    nc = _get_nc()

    x = np.asarray(x, dtype=np.float32)
    h0 = np.asarray(h0, dtype=np.float32)
    c0 = np.asarray(c0, dtype=np.float32)
    Ws = {n: np.asarray(a, dtype=np.float32)
          for n, a in zip(W_NAMES, (W_ii, W_hi, W_if_, W_hf, W_cf,
                                    W_ic, W_hc, W_io, W_ho, W_co))}
    (b_ii, b_hi, b_if_, b_hf, b_cf, b_ic, b_hc, b_io, b_ho, b_co) = [
        np.asarray(a, dtype=np.float32)
        for a in (b_ii, b_hi, b_if_, b_hf, b_cf, b_ic, b_hc, b_io, b_ho, b_co)
    ]

    # combined per-gate biases, packed [p, mt, gate]
    bias = np.stack(
        [
            (b_ii + b_hi).reshape(MT, P).T,
            (b_if_ + b_hf + b_cf).reshape(MT, P).T,
            (b_ic + b_hc).reshape(MT, P).T,
            (b_io + b_ho + b_co).reshape(MT, P).T,
        ],
        axis=2,
    ).astype(np.float32)

    w_packed = {}
    for n, W in Ws.items():
        n8 = CONFIG[n]
        nb = NK - n8
        if n8 > 0:
            w_packed[f"w8_{n}"] = _pack_w(W, F8, nb, NK, SW)
        if nb > 0:
            w_packed[f"w16_{n}"] = _pack_w(W, BF, 0, nb, SW)

    in_maps = []
    for core in range(N_CORES):
        s = slice(core * B, (core + 1) * B)
        m = {
            "c0T": _pack_actT(c0[s], np.float32),
            "bias": bias,
        }
        if NEED_X8 and not DERIVE_X8:
            m["xT8"] = _pack_actT(x[s], F8, SA)
        if NEED_X16:
            m["xT16"] = _pack_actT(x[s], BF, SA)
        if NEED_H8 and not DERIVE_H8:
            m["hT8"] = _pack_actT(h0[s], F8, SA)
        if NEED_H16:
            m["hT16"] = _pack_actT(h0[s], BF, SA)
        if NEED_C8:
            m["cT8"] = _pack_actT(c0[s], F8, SA)
        if NEED_C16:
            m["cT16"] = _pack_actT(c0[s], BF, SA)
        m.update(w_packed)
        in_maps.append(m)

    res = run_bass_kernel_spmd(nc, in_maps, list(range(N_CORES)), trace=_trace)

    o_g = np.empty((BATCH, HID), np.float32)
    h1 = np.empty((BATCH, HID), np.float32)
    c1 = np.empty((BATCH, HID), np.float32)
    for core in range(N_CORES):
        s = slice(core * B, (core + 1) * B)
        o_g[s] = _unpack_out(res.results[core]["ogT"])
        h1[s] = _unpack_out(res.results[core]["h1T"])
        c1[s] = _unpack_out(res.results[core]["c1T"])
    out = (o_g, h1, c1)
    if _trace:
        return out, res
    return out
